# revision 23
# baseline (speedup 1.0000x reference)
"""GPT-OSS expert MLP (gate/up GEMM + clamped GLU + down GEMM + routing scale)
on 8 Trainium2 NeuronCores.

Sharding: tensor-parallel split of the intermediate dim I=2880 across 8 cores
(360 columns each, padded to 384 = 3*128). Each core computes
  gate/up = hidden @ W[:, slice] ; glu ; y_partial = glu_h @ down_w[slice, :]
and writes its full [H, T] partial (transposed layout). The host sums the 8
partials, applies down bias, routing weights, and the residual add.

Tokens whose routing weight sum is exactly zero (all expert_mask slots 0)
contribute nothing to the output; they are dropped host-side and the device
program is compiled for the compacted token count (T_pad).

The device loop is software-pipelined (For_i_pipelined, 2 stages): the loads
of iteration i+1 (hid/gu/dw/bias, double-buffered) stream while iteration i
computes, so the steady-state period approaches max(PE, DMA) instead of
head + PE + tail per iteration.

All matmul operands are bf16: the quantized weights (values k/32, |k|<=4) are
exactly representable in bf16, so the only rounding is on hidden_states.
PSUM accumulation is fp32; partials are written out in bf16 and
summed on the host in fp64.
"""

import numpy as np
import ml_dtypes

BF16 = ml_dtypes.bfloat16

H = 2880          # hidden size
I = 2880          # intermediate size
NCORES = 8
IC = I // NCORES  # 360 intermediate cols per core
ICP = 384         # padded to 3 * 128
MT = ICP // 128   # 3 i-tiles per core
HP = 2944         # H padded to 23 * 128
KT = HP // 128    # 23 k-tiles over hidden dim
ALPHA = 1.702
LIMIT = 7.0
UNROLL = 2
_cache = {}


def build_program(loop_reps=None, T=512, unroll=UNROLL, staggered=False,
                  act_fn=None, skip_stores=False, skip_copies=False,
                  skip_loads=False):
    """Build (and compile) the per-core Bass program for T tokens. Identical
    on all cores; per-core data comes from in_maps. If loop_reps is given,
    the body runs loop_reps times under a software-pipelined device loop
    (used only for timing)."""
    import concourse.bacc as bacc
    import concourse.mybir as mybir
    import concourse.tile as tile

    fp32 = mybir.dt.float32
    bf16 = mybir.dt.bfloat16

    nc = bacc.Bacc("TRN2", target_bir_lowering=False, debug=False,
                   num_devices=NCORES)

    fp8 = mybir.dt.float8e4
    hid_d = nc.dram_tensor("hid", [128, KT * T], bf16, kind="ExternalInput").ap()
    # weights in HBM as fp8e4m3 — the quantized values (k/32, |k| <= 4) are
    # exactly representable; SWDGE casts to bf16 during the DMA, halving
    # the HBM read traffic for weights at no accuracy cost
    gu_d = nc.dram_tensor("gu", [128, 2 * MT * KT * 128], fp8,
                          kind="ExternalInput").ap()
    dw_d = nc.dram_tensor("dw", [128, KT * MT * 128], fp8,
                          kind="ExternalInput").ap()
    b_d = nc.dram_tensor("bias", [128, 2 * MT], fp32, kind="ExternalInput").ap()
    y_d = nc.dram_tensor("y", [HP, T], bf16, kind="ExternalOutput").ap()

    KT128 = KT * 128

    def make_stages(ctx, tc):
        glupool = ctx.enter_context(tc.tile_pool(name="glu", bufs=2))
        hglupool = ctx.enter_context(tc.tile_pool(name="hglu", bufs=2))
        ypool = ctx.enter_context(tc.tile_pool(name="yout", bufs=4))
        psum = ctx.enter_context(
            tc.tile_pool(name="psum", bufs=2, space="PSUM"))
        psum_y = ctx.enter_context(
            tc.tile_pool(name="psum_y", bufs=2, space="PSUM"))

        def load(pipe, iv):
            hid = pipe.intermediate_tile([128, KT * T], bf16, name="hid")
            gu = pipe.intermediate_tile([128, 2 * MT * KT128], bf16, name="gu")
            dw = pipe.intermediate_tile([128, KT * MT * 128], bf16, name="dw")
            bias = pipe.intermediate_tile([128, 2 * MT], fp32, name="bias")
            if skip_loads:  # timing probe: load only slivers
                nc.sync.dma_start(hid[:, :16], hid_d[:, :16])
                nc.gpsimd.dma_start(gu[:, :16], gu_d[:, :16])
                nc.gpsimd.dma_start(dw[:, :16], dw_d[:, :16])
            else:
                nc.sync.dma_start(hid[:], hid_d[:])
                # fp8 -> bf16 cast happens inline in the SDMA datapath
                nc.gpsimd.dma_start(gu[:], gu_d[:])
                nc.gpsimd.dma_start(dw[:], dw_d[:])
            nc.sync.dma_start(bias[:], b_d[:])
            return (hid, gu, dw, bias)

        def compute(pipe, iv, tiles):
            hid, gu, dw, bias = tiles
            hglu = hglupool.tile([128, MT * T], bf16, tag="hglu")

            # ---- gate/up GEMMs + GLU per i-tile ----
            for m in range(MT):
                pg = psum.tile([128, T], fp32, tag="pg")
                for kt in range(KT):
                    nc.tensor.matmul(
                        pg[:], gu[:, 2 * m * KT128 + kt * 128:
                                   2 * m * KT128 + (kt + 1) * 128],
                        hid[:, kt * T:(kt + 1) * T],
                        start=(kt == 0), stop=(kt == KT - 1))
                pu = psum.tile([128, T], fp32, tag="pu")
                for kt in range(KT):
                    nc.tensor.matmul(
                        pu[:], gu[:, (2 * m + 1) * KT128 + kt * 128:
                                   (2 * m + 1) * KT128 + (kt + 1) * 128],
                        hid[:, kt * T:(kt + 1) * T],
                        start=(kt == 0), stop=(kt == KT - 1))

                # gate path: g = min(pg + gb, LIMIT); sg = silu(ALPHA*g)
                tg = glupool.tile([128, T], fp32, tag="tg")
                nc.vector.tensor_scalar(tg[:], pg[:], bias[:, m:m + 1], LIMIT,
                                        mybir.AluOpType.add, mybir.AluOpType.min)
                sg = glupool.tile([128, T], fp32, tag="sg")
                nc.scalar.activation(sg[:], tg[:],
                                     act_fn or
                                     mybir.ActivationFunctionType.Silu,
                                     scale=ALPHA)
                # up path with +1 folded into the host-side bias (ub1 = ub+1):
                # u1 = clip(pu + ub1, 1-LIMIT, 1+LIMIT) = clip(up,-L,L) + 1
                tu = glupool.tile([128, T], fp32, tag="tu")
                nc.vector.tensor_scalar(tu[:], pu[:], bias[:, MT + m:MT + m + 1],
                                        1.0 + LIMIT,
                                        mybir.AluOpType.add, mybir.AluOpType.min)
                tu4 = glupool.tile([128, T], fp32, tag="tu4")
                nc.vector.tensor_scalar(tu4[:], tu[:], 1.0 - LIMIT, 1.0 / ALPHA,
                                        mybir.AluOpType.max, mybir.AluOpType.mult)
                # h = (ALPHA*glu) * (u+1)/ALPHA = glu * (u + 1)
                nc.vector.tensor_tensor(hglu[:, m * T:(m + 1) * T], sg[:],
                                        tu4[:], mybir.AluOpType.mult)

            # ---- down GEMM, write bf16 partial y^T in batched stores ----
            # h-tiles are processed in pairs sharing one 2-bank PSUM tile
            # (each matmul output stays inside its own 2 KiB bank); one
            # strided copy drains both banks, halving copy count and the
            # associated fixed overheads / semaphore traffic.
            # last batch kept small: at the loop-body barrier the final
            # store's copy+DMA tail sits on the critical path
            batches = [6, 6, 6, 4, 1]
            bidx = 0
            bi = 0
            yo = None
            nb = 0
            pyp = None
            for ht in range(KT):
                ph = ht % 2
                if ph == 0:
                    pyp = psum_y.tile([128, 1024], fp32, tag="py")
                py = pyp[:, 512 * ph: 512 * ph + T]
                for it in range(MT):
                    nc.tensor.matmul(
                        py,
                        dw[:, ht * ICP + it * 128: ht * ICP + (it + 1) * 128],
                        hglu[:, it * T:(it + 1) * T],
                        start=(it == 0), stop=(it == MT - 1))
                if skip_copies:
                    continue
                if bi == 0:
                    nb = batches[bidx]
                    yo = ypool.tile([128, nb * T], bf16, tag="yo")
                last = ht == KT - 1
                if ph == 1 or last:
                    npair = 1 if last else 2
                    src = pyp[:].rearrange("p (a x) -> p a x", a=2)[
                        :, :npair, :T]
                    dst = yo[:, (bi - npair + 1) * T:
                             (bi + 1) * T].rearrange(
                        "p (a x) -> p a x", a=npair)
                    # alternate PSUM->SBUF copies between DVE and ACT
                    if (ht // 2) % 2 == 0:
                        nc.vector.tensor_copy(dst, src)
                    else:
                        nc.scalar.copy(dst, src)
                bi += 1
                if bi == nb:
                    if not skip_stores:
                        h0 = ht - nb + 1
                        dsty = y_d[h0 * 128:(h0 + nb) * 128, :].rearrange(
                            "(a p) t -> p a t", p=128)
                        src_ap = yo[:].rearrange("p (a t) -> p a t", a=nb)
                        # stores on the ACT HWDGE ring: the SP ring carries
                        # the loads, which are issued in the pipelined tick
                        # order after compute[i]; a store on SP (data only
                        # ready late in compute[i]) would block the next
                        # loads behind it (FIFO head-of-line) and stall PE
                        # at the body boundary.
                        nc.scalar.dma_start(dsty, src_ap)
                    bidx += 1
                    bi = 0

        return [load, compute]

    from contextlib import ExitStack
    with tile.TileContext(nc) as tc:
        with ExitStack() as ctx:
            stages = make_stages(ctx, tc)
            reps = 1 if loop_reps is None else loop_reps
            tc.For_i_pipelined(stages, 0, reps, unroll=unroll,
                               staged_num_bufs=min(unroll, 2),
                               staggered_reset=staggered,
                               hint_engines=(mybir.EngineType.PE,))

    nc.compile()
    return nc


def _get_program(T, loop_reps=None, unroll=UNROLL, staggered=False):
    key = (T, loop_reps, unroll, staggered)
    if key not in _cache:
        _cache[key] = build_program(loop_reps=loop_reps, T=T, unroll=unroll,
                                    staggered=staggered)
    return _cache[key]


def token_weights(expert_mask, routing_weights):
    """Per-token scale: sum_j mask[j,t] * rw[t,j] (fp32, exact)."""
    mask = np.asarray(expert_mask, np.float32)          # [TOPK, T]
    rw = np.asarray(routing_weights, np.float32)        # [T, TOPK]
    return np.einsum("jt,tj->t", mask, rw)              # [T]


def pad_tokens(n):
    """Compiled token-count for n active tokens (multiple of 16, >= 32)."""
    return max(32, -(-n // 16) * 16)


def prepare_in_maps(hidden_states, gate_w, gate_b, up_w, up_b, down_w,
                    T=None):
    """Host-side shard + pad + pre-tile into the exact SBUF layouts.
    hidden_states may be pre-compacted; it is zero-padded to T rows."""
    from concourse import mybir
    F8 = mybir.dt.np(mybir.dt.float8e4)
    hs = np.asarray(hidden_states, np.float32)
    if T is None:
        T = pad_tokens(hs.shape[0])
    hidT = np.zeros((HP, T), np.float32)
    hidT[:H, :hs.shape[0]] = hs.T
    hid_tiled = np.ascontiguousarray(
        hidT.astype(BF16).reshape(KT, 128, T).transpose(1, 0, 2)
    ).reshape(128, KT * T)

    gw = np.asarray(gate_w, np.float32)
    uw = np.asarray(up_w, np.float32)
    dwf = np.asarray(down_w, np.float32)
    gbf = np.asarray(gate_b, np.float32).reshape(-1)
    ubf = np.asarray(up_b, np.float32).reshape(-1)

    def lhsT_tiles(Wp):  # [HP, 128] -> [128, KT*128]
        return np.ascontiguousarray(
            Wp.reshape(KT, 128, 128).transpose(1, 0, 2)).reshape(128, KT * 128)

    in_maps = []
    for c in range(NCORES):
        sl = slice(c * IC, (c + 1) * IC)
        Gp = np.zeros((HP, ICP), np.float32)
        Gp[:H, :IC] = gw[:, sl]
        Up = np.zeros((HP, ICP), np.float32)
        Up[:H, :IC] = uw[:, sl]
        Gp = Gp.astype(F8)
        Up = Up.astype(F8)
        blocks = []
        for m in range(MT):
            blocks.append(lhsT_tiles(Gp[:, m * 128:(m + 1) * 128]))
            blocks.append(lhsT_tiles(Up[:, m * 128:(m + 1) * 128]))
        gu = np.ascontiguousarray(np.concatenate(blocks, axis=1))

        Dp = np.zeros((ICP, HP), np.float32)
        Dp[:IC, :H] = dwf[sl, :]
        dw_tiled = np.ascontiguousarray(
            Dp.astype(F8).reshape(MT, 128, KT, 128).transpose(1, 2, 0, 3)
        ).reshape(128, KT * MT * 128)

        # bias layout [128, 2*MT]: cols 0..MT-1 gate bias, MT..2MT-1 = up+1
        bias = np.zeros((2 * MT, 128), np.float32)
        gbp = np.zeros(ICP, np.float32)
        gbp[:IC] = gbf[sl]
        ubp = np.zeros(ICP, np.float32)
        ubp[:IC] = ubf[sl] + 1.0
        bias[:MT] = gbp.reshape(MT, 128)
        bias[MT:] = ubp.reshape(MT, 128)

        in_maps.append({
            "hid": hid_tiled,
            "gu": gu,
            "dw": dw_tiled,
            "bias": np.ascontiguousarray(bias.T),
        })
    return in_maps


def kernel(hidden_states, routing_weights, final_hidden_states,
           gate_w, gate_b, up_w, up_b, down_w, down_b, expert_mask):
    from concourse.bass_utils import run_bass_kernel_spmd

    tok_w = token_weights(expert_mask, routing_weights)     # [T]
    sel = np.nonzero(tok_w != 0.0)[0]
    out = np.asarray(final_hidden_states, np.float32).copy()
    if sel.size == 0:
        return out.astype(np.float32)

    hs = np.asarray(hidden_states, np.float32)[sel]         # [T_act, H]
    T_pad = pad_tokens(sel.size)
    nc = _get_program(T_pad)
    in_maps = prepare_in_maps(hs, gate_w, gate_b, up_w, up_b, down_w, T=T_pad)
    res = run_bass_kernel_spmd(nc, in_maps, list(range(NCORES)))

    ysum = np.zeros((HP, T_pad), np.float64)
    for c in range(NCORES):
        ysum += res.results[c]["y"].astype(np.float64)
    y = ysum[:H, :sel.size].T.astype(np.float32)            # [T_act, H]

    out[sel] += ((y + np.asarray(down_b, np.float32).reshape(1, -1))
                 * tok_w[sel, None])
    return out.astype(np.float32)
